# revision 26
# baseline (speedup 1.0000x reference)
import sys

sys.path.insert(0, "/opt/trn_rl_repo")

import numpy as np

B, T_FEATS, T_TEXT = 16, 2000, 400
ADIM, ODIM = 256, 512
BIG_NEG = -1e30
BLANK_LOGP = -1.0

NCORES = 8
BPC = B // NCORES  # samples per core

CH = 464
CHUNKS = [(0, 384), (384, 384), (768, 384), (1152, 384), (1536, 464)]

_STATE = {}


# ---------------------------------------------------------------- device build
def _build_bass():
    import concourse.bass as bass
    import concourse.mybir as mybir
    import concourse.tile as tile
    from concourse import bacc

    f32 = mybir.dt.float32
    f32r = mybir.dt.float32r
    AF = mybir.ActivationFunctionType
    OP = mybir.AluOpType

    nc = bacc.Bacc("TRN2", target_bir_lowering=False, debug=False, num_devices=NCORES)

    xs = nc.dram_tensor("xs", (BPC, 128, 4, 2004), f32, kind="ExternalInput").ap()
    xt = nc.dram_tensor("xt", (BPC, 128, 2, 402), f32, kind="ExternalInput").ap()
    w1 = nc.dram_tensor("w1", (128, 3072), f32, kind="ExternalInput").ap()
    w2 = nc.dram_tensor("w2", (128, 1536), f32, kind="ExternalInput").ap()
    w3 = nc.dram_tensor("w3", (128, 512), f32, kind="ExternalInput").ap()
    wt1 = nc.dram_tensor("wt1", (128, 1536), f32, kind="ExternalInput").ap()
    wt2 = nc.dram_tensor("wt2", (128, 512), f32, kind="ExternalInput").ap()
    bia = nc.dram_tensor("bia", (128, 10), f32, kind="ExternalInput").ap()
    npr = nc.dram_tensor("npr", (4, 128, 4, 400), f32, kind="ExternalInput").ap()
    lpn = nc.dram_tensor("lpn", (BPC, 2000, 400), f32, kind="ExternalOutput").ap()

    def r(ap):
        return ap.bitcast(f32r)

    with TileCtx(tile, nc) as (tc, pools):
        consts = pools["consts"]
        psA = pools["psA"]
        psE = pools["psE"]
        xp = pools["xp"]
        act = pools["act"]
        sm = pools["sm"]
        npp = pools["npp"]

        # ---- resident weights (DMA f32 staging -> gpsimd round to f32r)
        stg = pools["stg"]

        def load_w(dram, n, tag):
            sb = consts.tile([128, n * 256], f32r, tag=tag, name=tag)
            s = stg.tile([128, n * 256], f32, tag="wstg", name=f"{tag}_stg")
            nc.sync.dma_start(s[:], dram[:])
            nc.gpsimd.tensor_scalar_add(sb[:], s[:], 0.0)
            return sb

        wt1sb = load_w(wt1, 6, "wt1sb")
        wt2sb = load_w(wt2, 2, "wt2sb")
        w1sb = load_w(w1, 12, "w1sb")
        w2sb = load_w(w2, 6, "w2sb")
        w3sb = load_w(w3, 2, "w3sb")

        btile = consts.tile([128, 10], f32, tag="btile")
        nc.sync.dma_start(btile[:], bia[:])
        bt = {}
        for bi, nm in enumerate(["fb1", "fb2", "fb3", "tb1", "tb2"]):
            for kt in range(2):
                bt[(nm, kt)] = btile[:, bi * 2 + kt : bi * 2 + kt + 1]

        eps_col = consts.tile([128, 1], f32, tag="eps_col")
        nc.vector.memset(eps_col[:], 1e-6)
        ones_f32 = consts.tile([128, CH], f32, tag="ones_f32")
        nc.vector.memset(ones_f32[:], 1.0)
        ones_col = consts.tile([128, 1], f32r, tag="ones_col")
        nc.gpsimd.tensor_scalar_add(ones_col[:], ones_f32[:, :1], 0.0)
        ones_rowL = consts.tile([1, CH], f32r, tag="ones_rowL")
        nc.gpsimd.tensor_scalar_add(ones_rowL[:], ones_f32[:1, :CH], 0.0)
        ones_rowN = consts.tile([1, 400], f32r, tag="ones_rowN")
        nc.gpsimd.tensor_scalar_add(ones_rowN[:], ones_f32[:1, :400], 0.0)

        # ---- t branch (per sample): produce tm2 (=-2*t) and trow ([tsq; ones])
        tm2 = {}
        trow = {}
        for s in range(BPC):
            s_ = stg.tile([128, 2, 402], f32, tag="xtstg")
            nc.sync.dma_start(s_[:], xt[s])
            xtk = consts.tile([128, 2, 402], f32r, tag=f"xt_{s}", name=f"xt_{s}")
            nc.gpsimd.tensor_scalar_add(xtk[:], s_[:], 0.0)
            xts = [xtk[:, 0], xtk[:, 1]]
            t1 = []
            for mt in range(2):
                ps = psA.tile([128, 400], f32, tag="convps")
                k = 0
                for d in range(3):
                    for kt in range(2):
                        lh = wt1sb[:, (d * 2 + kt) * 256 + mt * 128 :][:, :128]
                        nc.tensor.matmul(
                            ps[:], r(lh), r(xts[kt][:, d : d + 400]),
                            start=(k == 0), stop=(k == 5),
                        )
                        k += 1
                y = act.tile([128, 400], f32r, tag="t1sb")
                nc.vector.tensor_scalar(
                    y[:], ps[:], bt[("tb1", mt)], 0.0, OP.add, OP.max
                )
                t1.append(y)
            for mt in range(2):
                ps = psA.tile([128, 400], f32, tag="convps")
                for kt in range(2):
                    lh = wt2sb[:, kt * 256 + mt * 128 :][:, :128]
                    nc.tensor.matmul(
                        ps[:], r(lh), r(t1[kt][:]), start=(kt == 0), stop=(kt == 1)
                    )
                tm = consts.tile([128, 400], f32r, tag=f"tm2_{s}_{mt}")
                # tm2 = (ps + tb2) * -2
                nc.vector.tensor_scalar(
                    tm[:], ps[:], bt[("tb2", mt)], -2.0, OP.add, OP.mult
                )
                tm2[(s, mt)] = tm
            # tsq row: sum_c t^2 = sum_c tm2^2 / 4
            rowp = psE.tile([1, 400], f32, tag="eps")
            tsq = []
            for mt in range(2):
                q = act.tile([128, 400], f32r, tag="t1sb")
                nc.vector.tensor_tensor(
                    q[:], tm2[(s, mt)][:], tm2[(s, mt)][:], OP.mult
                )
                tsq.append(q)
            for mt in range(2):
                nc.tensor.matmul(
                    rowp[:], r(ones_col[:]), r(tsq[mt][:]),
                    start=(mt == 0), stop=(mt == 1),
                )
            tr = consts.tile([1, 400], f32r, tag=f"trow_{s}")
            nc.scalar.mul(tr[:], rowp[:], 0.25)
            trow[s] = tr

        # ---- f branch streamed: conv -> distance -> softmax, all ACT ops in
        # table-set 6 (exp/ln family; sqrt(x) = exp(0.5 ln x)) so no reloads.
        POS = []
        for c, (t0, L) in enumerate(CHUNKS):
            for st in range((L + 127) // 128):
                POS.append((t0 + st * 128, min(128, L - st * 128)))

        nptiles = {}
        for g in range(4):
            npt = npp.tile([128, 4, 400], f32, tag="npt", name=f"np_{g}")
            nc.gpsimd.dma_start(npt[:], npr[g])
            nptiles[g] = npt

        pidx = 0
        for c, (t0, L) in enumerate(CHUNKS):
            nsub = (L + 127) // 128
            otile = [
                sm.tile([128, 2, 400], f32, tag="o", name=f"o_{c}_{st}")
                for st in range(nsub)
            ]
            for s in range(BPC):
                s_ = xp.tile([128, 4, CH + 4], f32, tag="Xstg")
                nc.sync.dma_start(s_[:, :, : L + 4], xs[s, :, :, t0 : t0 + L + 4])
                xtile = xp.tile([128, 4, CH + 4], f32r, tag="X")
                nc.gpsimd.tensor_scalar_add(
                    xtile[:, :, : L + 4], s_[:, :, : L + 4], 0.0
                )
                xc = [xtile[:, kt] for kt in range(4)]
                # conv1 (k=3, 512->256) + relu : window L+2
                y1 = []
                for mt in range(2):
                    ps = psA.tile([128, CH + 2], f32, tag="convps")
                    k = 0
                    for d in range(3):
                        for kt in range(4):
                            lh = w1sb[:, (d * 4 + kt) * 256 + mt * 128 :][:, :128]
                            nc.tensor.matmul(
                                ps[:, : L + 2], r(lh), r(xc[kt][:, d : d + L + 2]),
                                start=(k == 0), stop=(k == 11),
                            )
                            k += 1
                    y = act.tile([128, CH + 2], f32r, tag="y1sb")
                    nc.vector.tensor_scalar(
                        y[:, : L + 2], ps[:, : L + 2], bt[("fb1", mt)], 0.0,
                        OP.add, OP.max,
                    )
                    y1.append(y)
                # conv2 (k=3, 256->256) + relu : window L
                y2 = []
                for mt in range(2):
                    ps = psA.tile([128, CH], f32, tag="convps")
                    k = 0
                    for d in range(3):
                        for kt in range(2):
                            lh = w2sb[:, (d * 2 + kt) * 256 + mt * 128 :][:, :128]
                            nc.tensor.matmul(
                                ps[:, :L], r(lh), r(y1[kt][:, d : d + L]),
                                start=(k == 0), stop=(k == 5),
                            )
                            k += 1
                    y = act.tile([128, CH], f32r, tag="y2sb")
                    nc.vector.tensor_scalar(
                        y[:, :L], ps[:, :L], bt[("fb2", mt)], 0.0, OP.add, OP.max
                    )
                    y2.append(y)
                # conv3 (k=1, 256->256) + bias
                fsb = []
                for mt in range(2):
                    ps = psA.tile([128, CH], f32, tag="convps")
                    for kt in range(2):
                        lh = w3sb[:, kt * 256 + mt * 128 :][:, :128]
                        nc.tensor.matmul(
                            ps[:, :L], r(lh), r(y2[kt][:, :L]),
                            start=(kt == 0), stop=(kt == 1),
                        )
                    y = act.tile([128, CH], f32r, tag="fsb")
                    nc.vector.tensor_scalar_add(y[:, :L], ps[:, :L], bt[("fb3", mt)])
                    fsb.append(y)
                # fsq row
                rowp = psE.tile([1, CH], f32, tag="eps")
                for mt in range(2):
                    q = act.tile([128, CH], f32r, tag="y1sb")
                    nc.vector.tensor_tensor(
                        q[:, :L], fsb[mt][:, :L], fsb[mt][:, :L], OP.mult
                    )
                    nc.tensor.matmul(
                        rowp[:, :L], r(ones_col[:]), r(q[:, :L]),
                        start=(mt == 0), stop=(mt == 1),
                    )
                f2 = act.tile([1, CH], f32r, tag="f2")
                nc.vector.tensor_scalar_add(f2[:, :L], rowp[:, :L], 0.0)
                # einsum subtiles -> streamed softmax (all table-set 6)
                for st in range(nsub):
                    M = min(128, L - st * 128)
                    p_global = pidx + st
                    ps = psE.tile([128, 400], f32, tag="eps")
                    nc.tensor.matmul(
                        ps[:M, :], r(fsb[0][:, st * 128 : st * 128 + M]),
                        r(tm2[(s, 0)][:]), start=True, stop=False,
                    )
                    nc.tensor.matmul(
                        ps[:M, :], r(fsb[1][:, st * 128 : st * 128 + M]),
                        r(tm2[(s, 1)][:]), start=False, stop=False,
                    )
                    nc.tensor.matmul(
                        ps[:M, :], r(ones_rowL[:, :M]),
                        r(trow[s][:]), start=False, stop=False,
                    )
                    nc.tensor.matmul(
                        ps[:M, :], r(f2[:, st * 128 : st * 128 + M]),
                        r(ones_rowN[:]), start=False, stop=True,
                    )
                    v = sm.tile([128, 400], f32, tag="v")
                    nc.scalar.activation(v[:M, :], ps[:M, :], AF.Ln,
                                         bias=eps_col[:M, :])
                    u = sm.tile([128, 400], f32, tag="u")
                    nc.scalar.activation(u[:M, :], v[:M, :], AF.Exp, scale=0.5)
                    e = sm.tile([128, 400], f32, tag="e")
                    z = sm.tile([128, 1], f32, tag="z")
                    nc.scalar.activation(
                        e[:M, :], u[:M, :], AF.Exp, scale=-1.0,
                        accum_out=z[:M, :],
                    )
                    lnz = sm.tile([128, 1], f32, tag="lnz")
                    nc.scalar.activation(lnz[:M, :], z[:M, :], AF.Ln)
                    npt = nptiles[p_global // 4][:, p_global % 4]
                    nc.vector.scalar_tensor_tensor(
                        otile[st][:M, s], u[:M, :], lnz[:M, :],
                        npt[:M], OP.add, OP.add,
                    )
            pidx += nsub
            for st in range(nsub):
                t0abs, M = POS[pidx - nsub + st]
                nc.gpsimd.dma_start(
                    lpn[:, t0abs : t0abs + M, :].rearrange("s r n -> r s n"),
                    otile[st][:M],
                )

    # Force all activations onto table set "natural_log_exp_and_others"
    # (exp+ln+copy+relu share one table -> a single load, no thrash).
    import concourse.bacc as bacc_mod
    orig_tables = bacc_mod.get_activation_tables

    def _only_set6(arch):
        t = orig_tables(arch)
        return {
            name: (funcs if name == "natural_log_exp_and_others" else set())
            for name, funcs in t.items()
        }

    bacc_mod.get_activation_tables = _only_set6
    try:
        nc.compile()
    finally:
        bacc_mod.get_activation_tables = orig_tables
    return nc


class TileCtx:
    """Context manager bundling TileContext + pools."""

    def __init__(self, tile_mod, nc):
        self.tile = tile_mod
        self.nc = nc

    def __enter__(self):
        from contextlib import ExitStack

        self.stack = ExitStack()
        tc = self.stack.enter_context(self.tile.TileContext(self.nc))
        pools = {
            "consts": self.stack.enter_context(tc.tile_pool(name="consts", bufs=1)),
            "psA": self.stack.enter_context(
                tc.tile_pool(name="psA", bufs=4, space="PSUM")
            ),
            "psE": self.stack.enter_context(
                tc.tile_pool(name="psE", bufs=4, space="PSUM")
            ),
            "xp": self.stack.enter_context(tc.tile_pool(name="xp", bufs=2)),
            "stg": self.stack.enter_context(tc.tile_pool(name="stg", bufs=2)),
            "act": self.stack.enter_context(tc.tile_pool(name="act", bufs=3)),
            "sm": self.stack.enter_context(tc.tile_pool(name="sm", bufs=3)),
            "npp": self.stack.enter_context(tc.tile_pool(name="npp", bufs=2)),
        }
        self.tc = tc
        return tc, pools

    def __exit__(self, *a):
        return self.stack.__exit__(*a)


# ---------------------------------------------------------------- host helpers
def _get_jax():
    import jax

    return jax, jax.devices("cpu")[0]


def _neg_prior_f32():
    """-_betabinom_prior(T_FEATS, T_TEXT), matching reference f32 arithmetic."""
    jax, cpu = _get_jax()
    import jax.numpy as jnp
    from jax.scipy.special import gammaln

    with jax.default_device(cpu):
        t = jnp.arange(1, T_FEATS + 1, dtype=jnp.float32)
        a = t[:, None]
        bb = (T_FEATS - t + 1.0)[:, None]
        k = jnp.arange(1, T_TEXT + 1, dtype=jnp.float32)[None, :]
        n = float(T_TEXT)
        betaln = lambda x, y: gammaln(x) + gammaln(y) - gammaln(x + y)
        logC = gammaln(n + 1.0) - gammaln(k + 1.0) - gammaln(n - k + 1.0)
        pr = logC + betaln(k + a, n - k + bb) - betaln(a, bb)
        return np.asarray(-pr, np.float32)


def _make_host_post():
    jax, cpu = _get_jax()
    import jax.numpy as jnp
    from jax import lax

    def post(log_p_attn):
        # ---- forwardsum (CTC)
        Bv, T, N = log_p_attn.shape
        lp = jnp.concatenate(
            [jnp.full((Bv, T, 1), BLANK_LOGP, log_p_attn.dtype), log_p_attn],
            axis=-1,
        )
        lp = jax.nn.log_softmax(lp, axis=-1)
        S = 2 * N + 1
        ext = np.zeros(S, np.int32)
        ext[1::2] = np.arange(1, N + 1)
        skip_ok = np.zeros(S, bool)
        skip_ok[3::2] = True
        ext_j = jnp.asarray(ext)
        skip_j = jnp.asarray(skip_ok)

        emit = lp[:, :, ext_j]
        alpha = jnp.full((Bv, S), BIG_NEG)
        alpha = alpha.at[:, 0].set(emit[:, 0, 0]).at[:, 1].set(emit[:, 0, 1])

        def step(alpha, e):
            a1 = jnp.concatenate(
                [jnp.full((Bv, 1), BIG_NEG), alpha[:, :-1]], axis=1
            )
            a2 = jnp.concatenate(
                [jnp.full((Bv, 2), BIG_NEG), alpha[:, :-2]], axis=1
            )
            a2 = jnp.where(skip_j[None, :], a2, BIG_NEG)
            alpha = jnp.logaddexp(jnp.logaddexp(alpha, a1), a2) + e
            return alpha, None

        alpha, _ = lax.scan(step, alpha, jnp.moveaxis(emit[:, 1:], 1, 0))
        nll = -jnp.logaddexp(alpha[:, -1], alpha[:, -2])
        forwardsum = jnp.mean(nll / N)

        # ---- MAS viterbi
        def mas_single(lpb):
            log_prob = lpb.T
            Nn, Tt = T_TEXT, T_FEATS
            q0 = jnp.where(jnp.arange(Nn) == 0, log_prob[0, 0], BIG_NEG)

            def fwd(q, col):
                q = (
                    jnp.maximum(
                        q, jnp.concatenate([jnp.full((1,), BIG_NEG), q[:-1]])
                    )
                    + col
                )
                return q, q

            _, qs = lax.scan(fwd, q0, log_prob[:, 1:].T)
            Q = lax.stop_gradient(jnp.concatenate([q0[None], qs], axis=0))

            def bwd(a_next, q_col):
                i_a = jnp.maximum(a_next - 1, 0)
                a = jnp.where(
                    a_next == 0,
                    0,
                    jnp.where(q_col[i_a] >= q_col[a_next], i_a, a_next),
                )
                a = a.astype(jnp.int32)
                return a, a

            _, A_rev = lax.scan(
                bwd, jnp.array(T_TEXT - 1, jnp.int32), Q[:-1], reverse=True
            )
            return jnp.concatenate([A_rev, jnp.array([T_TEXT - 1], jnp.int32)])

        A = jax.vmap(mas_single)(log_p_attn)
        ds = jax.vmap(lambda a: jnp.bincount(a, length=T_TEXT))(A)
        ds = ds.astype(log_p_attn.dtype)
        gathered = jnp.take_along_axis(log_p_attn, A[:, :, None], axis=2)[..., 0]
        bin_loss = -jnp.mean(jnp.mean(gathered, axis=1))
        return ds, bin_loss, forwardsum

    with jax.default_device(cpu):
        post_j = jax.jit(post)

    def run(lp_np):
        with jax.default_device(cpu):
            ds, bl, fs = post_j(lp_np)
            return (
                np.asarray(ds, np.float32),
                np.asarray(bl, np.float32),
                np.asarray(fs, np.float32),
            )

    return run


def _get_state():
    if "nc" not in _STATE:
        _STATE["nc"] = _build_bass()
        npf = _neg_prior_f32()  # (2000, 400)
        pad = np.zeros((2048, 400), np.float32)
        pad[:2000] = npf
        _STATE["negprior"] = np.ascontiguousarray(
            pad.reshape(4, 4, 128, 400).transpose(0, 2, 1, 3)
        )
        _STATE["post"] = _make_host_post()
    return _STATE


def _prep_inputs(speech, txt, tw1, tb1, tw2, tb2, fw1, fb1, fw2, fb2, fw3, fb3):
    f32 = np.float32
    xs = np.zeros((B, 128, 4, 2004), f32)
    xs[:, :, :, 2:2002] = (
        speech.transpose(0, 2, 1).reshape(B, 4, 128, 2000).transpose(0, 2, 1, 3)
    )
    xt = np.zeros((B, 128, 2, 402), f32)
    xt[:, :, :, 1:401] = (
        txt.transpose(0, 2, 1).reshape(B, 2, 128, 400).transpose(0, 2, 1, 3)
    )

    def wlay(w, nk):
        # (o, c, d) -> (128, d*nk*256 + o) with i = d*nk + kt
        n = w.shape[2] * nk
        a = w.transpose(2, 1, 0).reshape(w.shape[2], nk, 128, 256)
        return np.ascontiguousarray(
            a.transpose(2, 0, 1, 3).reshape(128, n * 256)
        ).astype(f32)

    w1 = wlay(fw1, 4)
    w2 = wlay(fw2, 2)
    w3 = wlay(fw3, 2)
    wt1 = wlay(tw1, 2)
    wt2 = wlay(tw2, 2)
    bia = np.stack(
        [np.asarray(x, f32).reshape(2, 128) for x in [fb1, fb2, fb3, tb1, tb2]]
    )  # (5, 2, 128)
    bia = np.ascontiguousarray(bia.reshape(10, 128).T)  # (128, 10)
    return xs, xt, w1, w2, w3, wt1, wt2, bia


def _run_cached(nc, in_maps):
    """run_bass_via_pjrt with the jitted executable cached across calls."""
    import jax
    import numpy as np
    from jax.sharding import Mesh, PartitionSpec
    from jax.experimental.shard_map import shard_map
    from concourse import bass2jax
    from concourse import mybir

    n_cores = len(in_maps)
    if "runner" not in _STATE:
        bass2jax.install_neuronx_cc_hook()
        partition_name = (
            nc.partition_id_tensor.name if nc.partition_id_tensor else None
        )
        in_names, out_names, out_avals, zero_shapes = [], [], [], []
        for alloc in nc.m.functions[0].allocations:
            if not isinstance(alloc, mybir.MemoryLocationSet):
                continue
            name = alloc.memorylocations[0].name
            if alloc.kind == "ExternalInput":
                if name != partition_name:
                    in_names.append(name)
            elif alloc.kind == "ExternalOutput":
                shape = tuple(alloc.tensor_shape)
                dtype = mybir.dt.np(alloc.dtype)
                out_avals.append(jax.core.ShapedArray(shape, dtype))
                out_names.append(name)
                zero_shapes.append((shape, dtype))
        n_params = len(in_names)
        n_outs = len(out_names)
        all_in_names = list(in_names) + list(out_names)
        if partition_name is not None:
            all_in_names.append(partition_name)

        def _body(*args):
            operands = list(args)
            if partition_name is not None:
                operands.append(bass2jax.partition_id_tensor())
            outs = bass2jax._bass_exec_p.bind(
                *operands,
                out_avals=tuple(out_avals),
                in_names=tuple(all_in_names),
                out_names=tuple(out_names),
                lowering_input_output_aliases=(),
                sim_require_finite=True,
                sim_require_nnan=True,
                nc=nc,
            )
            return tuple(outs)

        devices = jax.devices()[:n_cores]
        mesh = Mesh(np.asarray(devices), ("core",))
        in_specs = (PartitionSpec("core"),) * (n_params + n_outs)
        out_specs = (PartitionSpec("core"),) * n_outs
        donate = tuple(range(n_params, n_params + n_outs))
        sharded = jax.jit(
            shard_map(_body, mesh=mesh, in_specs=in_specs,
                      out_specs=out_specs, check_rep=False),
            donate_argnums=donate, keep_unused=True,
        )
        _STATE["runner"] = (sharded, in_names, out_names, out_avals, zero_shapes)
    sharded, in_names, out_names, out_avals, zero_shapes = _STATE["runner"]
    concat_in = [
        np.concatenate([np.asarray(m[name]) for m in in_maps], axis=0)
        for name in in_names
    ]
    concat_zeros = [
        np.zeros((n_cores * s[0], *s[1:]), d) for (s, d) in zero_shapes
    ]
    out_arrs = sharded(*concat_in, *concat_zeros)
    return [
        {
            name: np.asarray(out_arrs[i]).reshape(
                n_cores, *out_avals[i].shape
            )[c]
            for i, name in enumerate(out_names)
        }
        for c in range(n_cores)
    ]


def kernel(speech, txt, speech_len, txt_len,
           tw1, tb1, tw2, tb2, fw1, fb1, fw2, fb2, fw3, fb3,
           _trace=False):
    from concourse.bass_utils import run_bass_kernel_spmd

    st = _get_state()
    speech = np.asarray(speech, np.float32)
    txt = np.asarray(txt, np.float32)
    args = [np.asarray(a, np.float32) for a in
            (tw1, tb1, tw2, tb2, fw1, fb1, fw2, fb2, fw3, fb3)]
    xs, xt, w1, w2, w3, wt1, wt2, bia = _prep_inputs(
        speech, txt, args[0], args[1], args[2], args[3], args[4], args[5],
        args[6], args[7], args[8], args[9]
    )
    in_maps = []
    for i in range(NCORES):
        sl = slice(i * BPC, (i + 1) * BPC)
        in_maps.append(
            dict(
                xs=np.ascontiguousarray(xs[sl]),
                xt=np.ascontiguousarray(xt[sl]),
                w1=w1, w2=w2, w3=w3, wt1=wt1, wt2=wt2, bia=bia,
                npr=st["negprior"],
            )
        )
    try:
        results = _run_cached(st["nc"], in_maps)
    except Exception:
        _STATE.pop("runner", None)
        results = run_bass_kernel_spmd(
            st["nc"], in_maps, core_ids=list(range(NCORES))
        ).results
    lp = -np.concatenate([results[i]["lpn"] for i in range(NCORES)], axis=0)
    ds, bin_loss, forwardsum = st["post"](lp)
    return ds, bin_loss, forwardsum, lp


# revision 39
# speedup vs baseline: 1.0750x; 1.0750x over previous
import sys

sys.path.insert(0, "/opt/trn_rl_repo")

import numpy as np

B, T_FEATS, T_TEXT = 16, 2000, 400
ADIM, ODIM = 256, 512
BIG_NEG = -1e30
BLANK_LOGP = -1.0

NCORES = 8
BPC = B // NCORES  # samples per core

CH = 464
CHUNKS = [(0, 384), (384, 384), (768, 384), (1152, 384), (1536, 464)]

_STATE = {}


# ---------------------------------------------------------------- device build
def _build_bass():
    import concourse.bass as bass
    import concourse.mybir as mybir
    import concourse.tile as tile
    from concourse import bacc

    f32 = mybir.dt.float32
    f32r = mybir.dt.float32r
    AF = mybir.ActivationFunctionType
    OP = mybir.AluOpType

    nc = bacc.Bacc("TRN2", target_bir_lowering=False, debug=False, num_devices=NCORES)

    xs = nc.dram_tensor("xs", (BPC, 128, 4, 2004), f32, kind="ExternalInput").ap()
    xt = nc.dram_tensor("xt", (BPC, 128, 2, 402), f32, kind="ExternalInput").ap()
    w1 = nc.dram_tensor("w1", (128, 3072), f32, kind="ExternalInput").ap()
    w2 = nc.dram_tensor("w2", (128, 1536), f32, kind="ExternalInput").ap()
    w3 = nc.dram_tensor("w3", (128, 512), f32, kind="ExternalInput").ap()
    wt1 = nc.dram_tensor("wt1", (128, 1536), f32, kind="ExternalInput").ap()
    wt2 = nc.dram_tensor("wt2", (128, 512), f32, kind="ExternalInput").ap()
    bia = nc.dram_tensor("bia", (128, 10), f32, kind="ExternalInput").ap()
    npr = nc.dram_tensor("npr", (4, 128, 4, 400), f32, kind="ExternalInput").ap()
    lpn = nc.dram_tensor("lpn", (BPC, 2000, 400), f32, kind="ExternalOutput").ap()

    def r(ap):
        return ap.bitcast(f32r)

    with TileCtx(tile, nc) as (tc, pools):
        consts = pools["consts"]
        psA = pools["psA"]
        psE = pools["psE"]
        xp = pools["xp"]
        act = pools["act"]
        sm = pools["sm"]
        npp = pools["npp"]

        # ---- resident weights (DMA f32 staging -> gpsimd round to f32r)
        stg = pools["stg"]

        def load_w(dram, n, tag):
            sb = consts.tile([128, n * 256], f32r, tag=tag, name=tag)
            s = stg.tile([128, n * 256], f32, tag="wstg", name=f"{tag}_stg")
            nc.sync.dma_start(s[:], dram[:])
            nc.gpsimd.tensor_scalar_add(sb[:], s[:], 0.0)
            return sb

        wt1sb = load_w(wt1, 6, "wt1sb")
        xtks = []
        for s in range(BPC):
            s_ = stg.tile([128, 2, 402], f32, tag="xtstg", name=f"xts_{s}")
            nc.sync.dma_start(s_[:], xt[s])
            xtk = consts.tile([128, 2, 402], f32r, tag=f"xt_{s}", name=f"xt_{s}")
            nc.gpsimd.tensor_scalar_add(xtk[:], s_[:], 0.0)
            xtks.append(xtk)
        wt2sb = load_w(wt2, 2, "wt2sb")
        w1sb = load_w(w1, 12, "w1sb")
        w2sb = load_w(w2, 6, "w2sb")
        w3sb = load_w(w3, 2, "w3sb")

        btile = consts.tile([128, 10], f32, tag="btile")
        nc.sync.dma_start(btile[:], bia[:])
        bt = {}
        for bi, nm in enumerate(["fb1", "fb2", "fb3", "tb1", "tb2"]):
            for kt in range(2):
                bt[(nm, kt)] = btile[:, bi * 2 + kt : bi * 2 + kt + 1]

        eps_col = consts.tile([128, 1], f32, tag="eps_col")
        nc.vector.memset(eps_col[:], 1e-6)
        ones_f32 = consts.tile([128, CH], f32, tag="ones_f32")
        nc.vector.memset(ones_f32[:], 1.0)
        ones_col = consts.tile([128, 1], f32r, tag="ones_col")
        nc.gpsimd.tensor_scalar_add(ones_col[:], ones_f32[:, :1], 0.0)
        ones_col2 = consts.tile([128, 2], f32r, tag="ones_col2")
        nc.gpsimd.tensor_scalar_add(ones_col2[:], ones_f32[:, :2], 0.0)
        ones_rowL = consts.tile([1, CH], f32r, tag="ones_rowL")
        nc.gpsimd.tensor_scalar_add(ones_rowL[:], ones_f32[:1, :CH], 0.0)
        ones_rowN = consts.tile([1, 400], f32r, tag="ones_rowN")
        nc.gpsimd.tensor_scalar_add(ones_rowN[:], ones_f32[:1, :400], 0.0)

        # ---- t branch (per sample): produce tm2 (=-2*t) and trow ([tsq; ones])
        tm2 = {}
        trow = {}
        for s in range(BPC):
            xts = [xtks[s][:, 0], xtks[s][:, 1]]
            t1 = []
            for mt in range(2):
                ps = psA.tile([128, 400], f32, tag="convps")
                k = 0
                for d in range(3):
                    for kt in range(2):
                        lh = wt1sb[:, (d * 2 + kt) * 256 + mt * 128 :][:, :128]
                        nc.tensor.matmul(
                            ps[:], r(lh), r(xts[kt][:, d : d + 400]),
                            start=(k == 0), stop=(k == 5),
                        )
                        k += 1
                y = act.tile([128, 400], f32r, tag="t1sb")
                nc.vector.tensor_scalar(
                    y[:], ps[:], bt[("tb1", mt)], 0.0, OP.add, OP.max
                )
                t1.append(y)
            for mt in range(2):
                ps = psA.tile([128, 400], f32, tag="convps")
                for kt in range(2):
                    lh = wt2sb[:, kt * 256 + mt * 128 :][:, :128]
                    nc.tensor.matmul(
                        ps[:], r(lh), r(t1[kt][:]), start=(kt == 0), stop=(kt == 1)
                    )
                tm = consts.tile([128, 400], f32r, tag=f"tm2_{s}_{mt}")
                # tm2 = (ps + tb2) * -2
                nc.vector.tensor_scalar(
                    tm[:], ps[:], bt[("tb2", mt)], -2.0, OP.add, OP.mult
                )
                tm2[(s, mt)] = tm
            # tsq row: sum_c t^2 = sum_c tm2^2 / 4
            rowp = psE.tile([1, 400], f32, tag="eps")
            tsq = []
            for mt in range(2):
                q = act.tile([128, 400], f32r, tag="t1sb")
                nc.vector.tensor_tensor(
                    q[:], tm2[(s, mt)][:], tm2[(s, mt)][:], OP.mult
                )
                tsq.append(q)
            for mt in range(2):
                nc.tensor.matmul(
                    rowp[:], r(ones_col[:]), r(tsq[mt][:]),
                    start=(mt == 0), stop=(mt == 1),
                )
            tr = consts.tile([1, 400], f32r, tag=f"trow_{s}")
            nc.scalar.mul(tr[:], rowp[:], 0.25)
            trow[s] = tr

        # ---- f branch streamed: conv -> distance -> softmax, all ACT ops in
        # table-set 6 (exp/ln family; sqrt(x) = exp(0.5 ln x)) so no reloads.
        POS = []
        for c, (t0, L) in enumerate(CHUNKS):
            for st in range((L + 127) // 128):
                POS.append((t0 + st * 128, min(128, L - st * 128)))

        nptiles = {}
        for g in range(4):
            npt = npp.tile([128, 4, 400], f32, tag="npt", name=f"np_{g}")
            nc.gpsimd.dma_start(npt[:], npr[g])
            nptiles[g] = npt

        pidx = 0
        for c, (t0, L) in enumerate(CHUNKS):
            nsub = (L + 127) // 128
            otile = [
                sm.tile([128, 2, 400], f32, tag="o", name=f"o_{c}_{st}")
                for st in range(nsub)
            ]
            for s in range(BPC):
                s_ = xp.tile([128, 4, CH + 4], f32, tag="Xstg")
                nc.sync.dma_start(s_[:, :, : L + 4], xs[s, :, :, t0 : t0 + L + 4])
                xtile = xp.tile([128, 4, CH + 4], f32r, tag="X")
                nc.gpsimd.tensor_scalar_add(
                    xtile[:, :, : L + 4], s_[:, :, : L + 4], 0.0
                )
                xc = [xtile[:, kt] for kt in range(4)]
                # conv1 (k=3, 512->256) + relu : window L+2
                y1 = []
                for mt in range(2):
                    ps = psA.tile([128, CH + 2], f32, tag="convps")
                    k = 0
                    for d in range(3):
                        for kt in range(4):
                            lh = w1sb[:, (d * 4 + kt) * 256 + mt * 128 :][:, :128]
                            nc.tensor.matmul(
                                ps[:, : L + 2], r(lh), r(xc[kt][:, d : d + L + 2]),
                                start=(k == 0), stop=(k == 11),
                            )
                            k += 1
                    y = act.tile([128, CH + 2], f32r, tag="y1sb")
                    nc.vector.tensor_scalar(
                        y[:, : L + 2], ps[:, : L + 2], bt[("fb1", mt)], 0.0,
                        OP.add, OP.max,
                    )
                    y1.append(y)
                # conv2 (k=3, 256->256) + relu : window L
                y2 = []
                for mt in range(2):
                    ps = psA.tile([128, CH], f32, tag="convps")
                    k = 0
                    for d in range(3):
                        for kt in range(2):
                            lh = w2sb[:, (d * 2 + kt) * 256 + mt * 128 :][:, :128]
                            nc.tensor.matmul(
                                ps[:, :L], r(lh), r(y1[kt][:, d : d + L]),
                                start=(k == 0), stop=(k == 5),
                            )
                            k += 1
                    y = act.tile([128, CH], f32r, tag="y2sb")
                    nc.vector.tensor_scalar(
                        y[:, :L], ps[:, :L], bt[("fb2", mt)], 0.0, OP.add, OP.max
                    )
                    y2.append(y)
                # conv3 (k=1, 256->256) + bias
                fsb = []
                for mt in range(2):
                    ps = psA.tile([128, CH], f32, tag="convps")
                    for kt in range(2):
                        lh = w3sb[:, kt * 256 + mt * 128 :][:, :128]
                        nc.tensor.matmul(
                            ps[:, :L], r(lh), r(y2[kt][:, :L]),
                            start=(kt == 0), stop=(kt == 1),
                        )
                    y = act.tile([128, CH], f32r, tag="fsb")
                    nc.vector.tensor_scalar_add(y[:, :L], ps[:, :L], bt[("fb3", mt)])
                    fsb.append(y)
                # f^2 tiles; |f|^2 columns come per-subtile via N=1 matmuls
                qs = []
                for mt in range(2):
                    q = act.tile([128, CH], f32r, tag="y1sb")
                    nc.vector.tensor_tensor(
                        q[:, :L], fsb[mt][:, :L], fsb[mt][:, :L], OP.mult
                    )
                    qs.append(q)
                # einsum subtiles -> streamed softmax (all table-set 6)
                for st in range(nsub):
                    M = min(128, L - st * 128)
                    p_global = pidx + st
                    ps = psE.tile([128, 400], f32, tag="eps")
                    nc.tensor.matmul(
                        ps[:M, :], r(fsb[0][:, st * 128 : st * 128 + M]),
                        r(tm2[(s, 0)][:]), start=True, stop=False,
                    )
                    nc.tensor.matmul(
                        ps[:M, :], r(fsb[1][:, st * 128 : st * 128 + M]),
                        r(tm2[(s, 1)][:]), start=False, stop=False,
                    )
                    nc.tensor.matmul(
                        ps[:M, :], r(ones_rowL[:, :M]),
                        r(trow[s][:]), start=False, stop=True,
                    )
                    fq = psE.tile([128, 2], f32, tag="eps", name=f"fq_{c}_{s}_{st}")
                    for mt in range(2):
                        nc.tensor.matmul(
                            fq[:M, :], r(qs[mt][:, st * 128 : st * 128 + M]),
                            r(ones_col2[:]), start=(mt == 0), stop=(mt == 1),
                        )
                    fqe = sm.tile([128, 1], f32, tag="fqe")
                    nc.vector.tensor_scalar_add(fqe[:M, :], fq[:M, 0:1], 1e-6)
                    v = sm.tile([128, 400], f32, tag="v")
                    nc.scalar.activation(v[:M, :], ps[:M, :], AF.Ln,
                                         bias=fqe[:M, :])
                    u = sm.tile([128, 400], f32, tag="u")
                    nc.scalar.activation(u[:M, :], v[:M, :], AF.Exp, scale=0.5)
                    e = sm.tile([128, 400], f32, tag="e")
                    z = sm.tile([128, 1], f32, tag="z")
                    nc.scalar.activation(
                        e[:M, :], u[:M, :], AF.Exp, scale=-1.0,
                        accum_out=z[:M, :],
                    )
                    lnz = sm.tile([128, 1], f32, tag="lnz")
                    nc.scalar.activation(lnz[:M, :], z[:M, :], AF.Ln)
                    npt = nptiles[p_global // 4][:, p_global % 4]
                    nc.vector.scalar_tensor_tensor(
                        otile[st][:M, s], u[:M, :], lnz[:M, :],
                        npt[:M], OP.add, OP.add,
                    )
            pidx += nsub
            for st in range(nsub):
                t0abs, M = POS[pidx - nsub + st]
                nc.scalar.dma_start(
                    lpn[:, t0abs : t0abs + M, :].rearrange("s r n -> r s n"),
                    otile[st][:M],
                )

    # Force all activations onto table set "natural_log_exp_and_others"
    # (exp+ln+copy+relu share one table -> a single load, no thrash).
    import concourse.bacc as bacc_mod
    orig_tables = bacc_mod.get_activation_tables

    def _only_set6(arch):
        t = orig_tables(arch)
        return {
            name: (funcs if name == "natural_log_exp_and_others" else set())
            for name, funcs in t.items()
        }

    bacc_mod.get_activation_tables = _only_set6
    try:
        nc.compile()
    finally:
        bacc_mod.get_activation_tables = orig_tables
    return nc


class TileCtx:
    """Context manager bundling TileContext + pools."""

    def __init__(self, tile_mod, nc):
        self.tile = tile_mod
        self.nc = nc

    def __enter__(self):
        from contextlib import ExitStack

        self.stack = ExitStack()
        tc = self.stack.enter_context(self.tile.TileContext(self.nc))
        pools = {
            "consts": self.stack.enter_context(tc.tile_pool(name="consts", bufs=1)),
            "psA": self.stack.enter_context(
                tc.tile_pool(name="psA", bufs=4, space="PSUM")
            ),
            "psE": self.stack.enter_context(
                tc.tile_pool(name="psE", bufs=4, space="PSUM")
            ),
            "xp": self.stack.enter_context(tc.tile_pool(name="xp", bufs=3)),
            "stg": self.stack.enter_context(tc.tile_pool(name="stg", bufs=2)),
            "act": self.stack.enter_context(tc.tile_pool(name="act", bufs=4)),
            "sm": self.stack.enter_context(tc.tile_pool(name="sm", bufs=4)),
            "npp": self.stack.enter_context(tc.tile_pool(name="npp", bufs=2)),
        }
        self.tc = tc
        return tc, pools

    def __exit__(self, *a):
        return self.stack.__exit__(*a)


# ---------------------------------------------------------------- host helpers
def _get_jax():
    import jax

    return jax, jax.devices("cpu")[0]


def _neg_prior_f32():
    """-_betabinom_prior(T_FEATS, T_TEXT), matching reference f32 arithmetic."""
    jax, cpu = _get_jax()
    import jax.numpy as jnp
    from jax.scipy.special import gammaln

    with jax.default_device(cpu):
        t = jnp.arange(1, T_FEATS + 1, dtype=jnp.float32)
        a = t[:, None]
        bb = (T_FEATS - t + 1.0)[:, None]
        k = jnp.arange(1, T_TEXT + 1, dtype=jnp.float32)[None, :]
        n = float(T_TEXT)
        betaln = lambda x, y: gammaln(x) + gammaln(y) - gammaln(x + y)
        logC = gammaln(n + 1.0) - gammaln(k + 1.0) - gammaln(n - k + 1.0)
        pr = logC + betaln(k + a, n - k + bb) - betaln(a, bb)
        return np.asarray(-pr, np.float32)


def _make_host_post():
    jax, cpu = _get_jax()
    import jax.numpy as jnp
    from jax import lax

    def post(log_p_attn):
        # ---- forwardsum (CTC)
        Bv, T, N = log_p_attn.shape
        lp = jnp.concatenate(
            [jnp.full((Bv, T, 1), BLANK_LOGP, log_p_attn.dtype), log_p_attn],
            axis=-1,
        )
        lp = jax.nn.log_softmax(lp, axis=-1)
        S = 2 * N + 1
        ext = np.zeros(S, np.int32)
        ext[1::2] = np.arange(1, N + 1)
        skip_ok = np.zeros(S, bool)
        skip_ok[3::2] = True
        ext_j = jnp.asarray(ext)
        skip_j = jnp.asarray(skip_ok)

        emit = lp[:, :, ext_j]
        alpha = jnp.full((Bv, S), BIG_NEG)
        alpha = alpha.at[:, 0].set(emit[:, 0, 0]).at[:, 1].set(emit[:, 0, 1])

        def step(alpha, e):
            a1 = jnp.concatenate(
                [jnp.full((Bv, 1), BIG_NEG), alpha[:, :-1]], axis=1
            )
            a2 = jnp.concatenate(
                [jnp.full((Bv, 2), BIG_NEG), alpha[:, :-2]], axis=1
            )
            a2 = jnp.where(skip_j[None, :], a2, BIG_NEG)
            alpha = jnp.logaddexp(jnp.logaddexp(alpha, a1), a2) + e
            return alpha, None

        alpha, _ = lax.scan(step, alpha, jnp.moveaxis(emit[:, 1:], 1, 0))
        nll = -jnp.logaddexp(alpha[:, -1], alpha[:, -2])
        forwardsum = jnp.mean(nll / N)

        # ---- MAS viterbi
        def mas_single(lpb):
            log_prob = lpb.T
            Nn, Tt = T_TEXT, T_FEATS
            q0 = jnp.where(jnp.arange(Nn) == 0, log_prob[0, 0], BIG_NEG)

            def fwd(q, col):
                q = (
                    jnp.maximum(
                        q, jnp.concatenate([jnp.full((1,), BIG_NEG), q[:-1]])
                    )
                    + col
                )
                return q, q

            _, qs = lax.scan(fwd, q0, log_prob[:, 1:].T)
            Q = lax.stop_gradient(jnp.concatenate([q0[None], qs], axis=0))

            def bwd(a_next, q_col):
                i_a = jnp.maximum(a_next - 1, 0)
                a = jnp.where(
                    a_next == 0,
                    0,
                    jnp.where(q_col[i_a] >= q_col[a_next], i_a, a_next),
                )
                a = a.astype(jnp.int32)
                return a, a

            _, A_rev = lax.scan(
                bwd, jnp.array(T_TEXT - 1, jnp.int32), Q[:-1], reverse=True
            )
            return jnp.concatenate([A_rev, jnp.array([T_TEXT - 1], jnp.int32)])

        A = jax.vmap(mas_single)(log_p_attn)
        ds = jax.vmap(lambda a: jnp.bincount(a, length=T_TEXT))(A)
        ds = ds.astype(log_p_attn.dtype)
        gathered = jnp.take_along_axis(log_p_attn, A[:, :, None], axis=2)[..., 0]
        bin_loss = -jnp.mean(jnp.mean(gathered, axis=1))
        return ds, bin_loss, forwardsum

    with jax.default_device(cpu):
        post_j = jax.jit(post)

    def run(lp_np):
        with jax.default_device(cpu):
            ds, bl, fs = post_j(lp_np)
            return (
                np.asarray(ds, np.float32),
                np.asarray(bl, np.float32),
                np.asarray(fs, np.float32),
            )

    return run


def _get_state():
    if "nc" not in _STATE:
        _STATE["nc"] = _build_bass()
        npf = _neg_prior_f32()  # (2000, 400)
        pad = np.zeros((2048, 400), np.float32)
        pad[:2000] = npf
        _STATE["negprior"] = np.ascontiguousarray(
            pad.reshape(4, 4, 128, 400).transpose(0, 2, 1, 3)
        )
        _STATE["post"] = _make_host_post()
    return _STATE


def _prep_inputs(speech, txt, tw1, tb1, tw2, tb2, fw1, fb1, fw2, fb2, fw3, fb3):
    f32 = np.float32
    xs = np.zeros((B, 128, 4, 2004), f32)
    xs[:, :, :, 2:2002] = (
        speech.transpose(0, 2, 1).reshape(B, 4, 128, 2000).transpose(0, 2, 1, 3)
    )
    xt = np.zeros((B, 128, 2, 402), f32)
    xt[:, :, :, 1:401] = (
        txt.transpose(0, 2, 1).reshape(B, 2, 128, 400).transpose(0, 2, 1, 3)
    )

    def wlay(w, nk):
        # (o, c, d) -> (128, d*nk*256 + o) with i = d*nk + kt
        n = w.shape[2] * nk
        a = w.transpose(2, 1, 0).reshape(w.shape[2], nk, 128, 256)
        return np.ascontiguousarray(
            a.transpose(2, 0, 1, 3).reshape(128, n * 256)
        ).astype(f32)

    w1 = wlay(fw1, 4)
    w2 = wlay(fw2, 2)
    w3 = wlay(fw3, 2)
    wt1 = wlay(tw1, 2)
    wt2 = wlay(tw2, 2)
    bia = np.stack(
        [np.asarray(x, f32).reshape(2, 128) for x in [fb1, fb2, fb3, tb1, tb2]]
    )  # (5, 2, 128)
    bia = np.ascontiguousarray(bia.reshape(10, 128).T)  # (128, 10)
    return xs, xt, w1, w2, w3, wt1, wt2, bia


def _run_cached(nc, in_maps):
    """run_bass_via_pjrt with the jitted executable cached across calls."""
    import jax
    import numpy as np
    from jax.sharding import Mesh, PartitionSpec
    from jax.experimental.shard_map import shard_map
    from concourse import bass2jax
    from concourse import mybir

    n_cores = len(in_maps)
    if "runner" not in _STATE:
        bass2jax.install_neuronx_cc_hook()
        partition_name = (
            nc.partition_id_tensor.name if nc.partition_id_tensor else None
        )
        in_names, out_names, out_avals, zero_shapes = [], [], [], []
        for alloc in nc.m.functions[0].allocations:
            if not isinstance(alloc, mybir.MemoryLocationSet):
                continue
            name = alloc.memorylocations[0].name
            if alloc.kind == "ExternalInput":
                if name != partition_name:
                    in_names.append(name)
            elif alloc.kind == "ExternalOutput":
                shape = tuple(alloc.tensor_shape)
                dtype = mybir.dt.np(alloc.dtype)
                out_avals.append(jax.core.ShapedArray(shape, dtype))
                out_names.append(name)
                zero_shapes.append((shape, dtype))
        n_params = len(in_names)
        n_outs = len(out_names)
        all_in_names = list(in_names) + list(out_names)
        if partition_name is not None:
            all_in_names.append(partition_name)

        def _body(*args):
            operands = list(args)
            if partition_name is not None:
                operands.append(bass2jax.partition_id_tensor())
            outs = bass2jax._bass_exec_p.bind(
                *operands,
                out_avals=tuple(out_avals),
                in_names=tuple(all_in_names),
                out_names=tuple(out_names),
                lowering_input_output_aliases=(),
                sim_require_finite=True,
                sim_require_nnan=True,
                nc=nc,
            )
            return tuple(outs)

        devices = jax.devices()[:n_cores]
        mesh = Mesh(np.asarray(devices), ("core",))
        in_specs = (PartitionSpec("core"),) * (n_params + n_outs)
        out_specs = (PartitionSpec("core"),) * n_outs
        donate = tuple(range(n_params, n_params + n_outs))
        sharded = jax.jit(
            shard_map(_body, mesh=mesh, in_specs=in_specs,
                      out_specs=out_specs, check_rep=False),
            donate_argnums=donate, keep_unused=True,
        )
        _STATE["runner"] = (sharded, in_names, out_names, out_avals, zero_shapes)
    sharded, in_names, out_names, out_avals, zero_shapes = _STATE["runner"]
    # Device-cache inputs whose content is stable across calls (weights,
    # prior) to avoid re-uploading ~60MB per call. Guarded by content hash.
    import hashlib
    from jax.sharding import Mesh as _Mesh, PartitionSpec as _P, NamedSharding
    static = {"w1", "w2", "w3", "wt1", "wt2", "bia", "npr"}
    dev_cache = _STATE.setdefault("dev_cache", {})
    mesh = _STATE.setdefault(
        "mesh", _Mesh(np.asarray(jax.devices()[:n_cores]), ("core",))
    )
    sh = NamedSharding(mesh, _P("core"))
    concat_in = []
    for name in in_names:
        arr = np.concatenate([np.asarray(m[name]) for m in in_maps], axis=0)
        if name in static:
            h = hashlib.md5(arr.tobytes()).hexdigest()
            ent = dev_cache.get(name)
            if ent is None or ent[0] != h:
                ent = (h, jax.device_put(arr, sh))
                dev_cache[name] = ent
            concat_in.append(ent[1])
        else:
            concat_in.append(arr)
    concat_zeros = [
        np.zeros((n_cores * s[0], *s[1:]), d) for (s, d) in zero_shapes
    ]
    out_arrs = sharded(*concat_in, *concat_zeros)
    return [
        {
            name: np.asarray(out_arrs[i]).reshape(
                n_cores, *out_avals[i].shape
            )[c]
            for i, name in enumerate(out_names)
        }
        for c in range(n_cores)
    ]


def kernel(speech, txt, speech_len, txt_len,
           tw1, tb1, tw2, tb2, fw1, fb1, fw2, fb2, fw3, fb3,
           _trace=False):
    from concourse.bass_utils import run_bass_kernel_spmd

    st = _get_state()
    speech = np.asarray(speech, np.float32)
    txt = np.asarray(txt, np.float32)
    args = [np.asarray(a, np.float32) for a in
            (tw1, tb1, tw2, tb2, fw1, fb1, fw2, fb2, fw3, fb3)]
    xs, xt, w1, w2, w3, wt1, wt2, bia = _prep_inputs(
        speech, txt, args[0], args[1], args[2], args[3], args[4], args[5],
        args[6], args[7], args[8], args[9]
    )
    in_maps = []
    for i in range(NCORES):
        sl = slice(i * BPC, (i + 1) * BPC)
        in_maps.append(
            dict(
                xs=np.ascontiguousarray(xs[sl]),
                xt=np.ascontiguousarray(xt[sl]),
                w1=w1, w2=w2, w3=w3, wt1=wt1, wt2=wt2, bia=bia,
                npr=st["negprior"],
            )
        )
    try:
        results = _run_cached(st["nc"], in_maps)
    except Exception:
        _STATE.pop("runner", None)
        results = run_bass_kernel_spmd(
            st["nc"], in_maps, core_ids=list(range(NCORES))
        ).results
    lp = -np.concatenate([results[i]["lpn"] for i in range(NCORES)], axis=0)
    ds, bin_loss, forwardsum = st["post"](lp)
    return ds, bin_loss, forwardsum, lp
